# revision 1
# baseline (speedup 1.0000x reference)
"""V6 (deep-buffered, partition-major DMA layouts + grouped ACT sets): all-bf16 inputs, batched 3D-window DMAs, wide fused ops.

Differences vs V1:
- tpad and x are passed as bf16 (t is exactly representable; x rounding adds
  ~1e-6 statistical noise to the mean).
- The row-aligned t needed for the sign-flip is a second DMA *view* of tpad
  (offset by the 2-row/2-col pad), so no shift matmul and no PSUM pressure.
- DMAs are batched: one 3D-AP transfer covers all 4 main windows of an image
  (and one covers the 4 tail windows of all 4 images).
- Elementwise/ACT ops run 2-window wide (1024 free-dim) or 4-window wide
  (softplus: 2048) to amortize fixed costs; the whole y path is uint16:
  y = (t_u16 << 8) XOR x_u16  (bf16 1.0 == 0x3F80 -> 0x8000 sign bit).
"""

import numpy as np

import concourse.bass as bass
import concourse.bacc as bacc_mod
import concourse.tile as tile
from concourse import mybir
from concourse.bass_utils import run_bass_kernel_spmd
from concourse.tile import add_dep_helper

F32 = mybir.dt.float32
BF16 = mybir.dt.bfloat16
U16 = mybir.dt.uint16
ALU = mybir.AluOpType
ACTF = mybir.ActivationFunctionType

B, H, W = 32, 512, 512
NCORES = 8
IMGS = B // NCORES
PAD = 2
TP = H + 2 * PAD            # 516
NWIN = 5
# main windows (4): t rows [124w, 124w+128), out rows [124w, 124w+124) at
# partitions [0,124).  tail: t rows [388, 516), out rows [496, 512) at
# partitions [0,16).
NMAIN = 4
TAIL_IS = 388
TAIL_NPW = 16
TAIL_G0 = 496


def _make_bands() -> np.ndarray:
    bands = np.zeros((2, 128, 124), dtype=np.float32)
    for m in range(124):
        bands[0, m: m + 5, m] = 1.0
    for m in range(16):
        bands[1, 108 + m: 113 + m, m] = 1.0
    return bands


def _ap3(t, off, dims):
    return bass.AP(t, off, dims)


def _build_nc() -> bass.Bass:
    nc = bacc_mod.Bacc(trn_type="TRN2")

    # host pre-arranged, partition-major inputs (contiguous per partition)
    twin = nc.dram_tensor("twin", [IMGS, 128, NWIN, TP], BF16, kind="ExternalInput")
    xw = nc.dram_tensor("xw", [IMGS, 124, NMAIN, W], BF16, kind="ExternalInput")
    taw = nc.dram_tensor("taw", [IMGS, 124, NMAIN, W], BF16, kind="ExternalInput")
    xtail = nc.dram_tensor("xtail", [IMGS, TAIL_NPW, W], BF16, kind="ExternalInput")
    tatail = nc.dram_tensor("tatail", [IMGS, TAIL_NPW, W], BF16, kind="ExternalInput")
    band = nc.dram_tensor("band", [2, 128, 124], BF16, kind="ExternalInput")
    out_sp = nc.dram_tensor("out_sp", [128, IMGS + 1], F32, kind="ExternalOutput")
    out_r = nc.dram_tensor("out_r", [128, 2 * IMGS + 2], F32, kind="ExternalOutput")

    IMG = TP * TP           # elements per padded image
    XIMG = H * W

    with tile.TileContext(nc) as tc:
        with (
            tc.tile_pool(name="singles", bufs=1) as singles,
            tc.tile_pool(name="tin", bufs=2) as tin,
            tc.tile_pool(name="xin", bufs=3) as xin,
            tc.tile_pool(name="tain", bufs=3) as tain,
            tc.tile_pool(name="vp", bufs=2) as vp,
            tc.tile_pool(name="yp", bufs=3) as yp,
            tc.tile_pool(name="spp", bufs=3) as spp,
            tc.tile_pool(name="eyp", bufs=2) as eyp,
            tc.tile_pool(name="uap", bufs=3) as uap,
            tc.tile_pool(name="scrp", bufs=3) as scrp,
            tc.tile_pool(name="mkp", bufs=3) as mkp,
            tc.tile_pool(name="psum", bufs=4, space="PSUM") as psum,
        ):
            band_sb = singles.tile([128, 2 * 124], BF16)
            nc.sync.dma_start(band_sb[:, 0:124], band[0])
            nc.sync.dma_start(band_sb[:, 124:248], band[1])
            bias_abs = singles.tile([128, 1], F32)
            nc.gpsimd.memset(bias_abs[:], -12.5)

            # integer shift amount for the bitvec stt (imm floats are rejected)
            shift8 = singles.tile([128, 1], U16)
            nc.vector.memset(shift8[:], 8)

            stats_sp = singles.tile([128, IMGS + 1], F32)
            stats_r = singles.tile([128, 2 * IMGS + 2], F32)
            nc.vector.memset(stats_sp[:], 0.0)
            nc.vector.memset(stats_r[:], 0.0)

            v_tl_t = singles.tile([128, IMGS, TP - 2], BF16)

            # ---- tail input tiles, batched across the 4 images ----
            t_tl = singles.tile([128, IMGS, TP], BF16)
            nc.sync.dma_start(
                t_tl[:],
                _ap3(twin, 4 * TP,
                     [[NWIN * TP, 128], [128 * NWIN * TP, IMGS], [1, TP]]),
            )
            x_tl = singles.tile([TAIL_NPW, IMGS, W], BF16)
            nc.sync.dma_start(
                x_tl[:],
                _ap3(xtail, 0, [[W, TAIL_NPW], [TAIL_NPW * W, IMGS], [1, W]]),
            )
            ta_tl = singles.tile([TAIL_NPW, IMGS, W], BF16)
            nc.sync.dma_start(
                ta_tl[:],
                _ap3(tatail, 0, [[W, TAIL_NPW], [TAIL_NPW * W, IMGS], [1, W]]),
            )

            # persistent cross-phase tiles (phase 2 reads them)
            t_all = singles.tile([128, IMGS, NMAIN, TP], BF16)
            v_all = singles.tile([128, IMGS, NMAIN, TP - 2], BF16)
            ey_all = singles.tile([124, IMGS, NMAIN, W], F32)
            ey_tl = singles.tile([TAIL_NPW, IMGS, W], F32)

            exp_insts = []

            # ---- phase 1: loads, box pair-sums, sign-flip, EXP (one ACT set)
            for im in range(IMGS):
                t_w = t_all[:, im, :, :]
                nc.sync.dma_start(
                    t_w,
                    _ap3(twin, im * 128 * NWIN * TP,
                         [[NWIN * TP, 128], [TP, NMAIN], [1, TP]]),
                )
                x_w = xin.tile([124, NMAIN, W], BF16, tag="x")
                nc.sync.dma_start(
                    x_w[:],
                    _ap3(xw, im * 124 * NMAIN * W,
                         [[NMAIN * W, 124], [W, NMAIN], [1, W]]),
                )
                ta_w = tain.tile([124, NMAIN, W], BF16, tag="ta")
                nc.sync.dma_start(
                    ta_w[:],
                    _ap3(taw, im * 124 * NMAIN * W,
                         [[NMAIN * W, 124], [W, NMAIN], [1, W]]),
                )

                nc.vector.tensor_tensor(
                    v_all[:, im, :, :], t_w[:, :, 0: TP - 2], t_w[:, :, 2:TP],
                    op=ALU.add,
                )

                mk = mkp.tile([124, NMAIN, W], BF16, tag="mk")
                nc.vector.tensor_scalar(
                    mk[:].bitcast(U16),
                    ta_w[:].bitcast(U16),
                    8,
                    None,
                    op0=ALU.logical_shift_left,
                )
                y_w = yp.tile([124, NMAIN, W], BF16, tag="y")
                nc.vector.tensor_tensor(
                    y_w[:].bitcast(U16),
                    mk[:].bitcast(U16),
                    x_w[:].bitcast(U16),
                    op=ALU.bitwise_xor,
                )
                exp_insts.append(nc.scalar.activation(ey_all[:, im, :, :], y_w[:], ACTF.Exp))

            # phase-1 tail: sign-flip + EXP while the exp set is loaded
            nc.vector.tensor_tensor(
                v_tl_t[:], t_tl[:, :, 0: TP - 2], t_tl[:, :, 2:TP], op=ALU.add
            )
            mk_tl = mkp.tile([TAIL_NPW, IMGS, W], BF16, tag="mk")
            nc.vector.tensor_scalar(
                mk_tl[:].bitcast(U16),
                ta_tl[:].bitcast(U16),
                8,
                None,
                op0=ALU.logical_shift_left,
            )
            y_tl = yp.tile([TAIL_NPW, IMGS, W], BF16, tag="y")
            nc.vector.tensor_tensor(
                y_tl[:].bitcast(U16),
                mk_tl[:].bitcast(U16),
                x_tl[:].bitcast(U16),
                op=ALU.bitwise_xor,
            )
            exp_insts.append(nc.scalar.activation(ey_tl[:], y_tl[:], ACTF.Exp))

            # ---- phase 2: LN (+accum), box matmuls, ABS, weighted term
            # (ln and abs share the natural_log set -> no more table loads)
            for im in range(IMGS):
                spy_w = spp.tile([124, NMAIN, W], BF16, tag="spy")
                ln_i = nc.scalar.activation(
                    spy_w[:],
                    ey_all[:, im, :, :],
                    ACTF.Ln,
                    bias=1.0,
                    accum_out=stats_sp[0:124, im: im + 1],
                )
                for g in range(NMAIN // 2):
                    s_ps = psum.tile([128, 2, W], F32, tag="s")
                    for j in range(2):
                        wgt = band_sb[:, 0:124]
                        wv = v_all[:, im, 2 * g + j, :]
                        wt = t_all[:, im, 2 * g + j, :]
                        nc.tensor.matmul(
                            s_ps[0:124, j, :], wgt, wv[:, 0:W],
                            start=True, stop=False,
                        )
                        nc.tensor.matmul(
                            s_ps[0:124, j, :], wgt, wv[:, 1: W + 1],
                            start=False, stop=False,
                        )
                        nc.tensor.matmul(
                            s_ps[0:124, j, :], wgt, wt[:, 4: W + 4],
                            start=False, stop=True,
                        )

                    uab = uap.tile([124, 2, W], BF16, tag="uab")
                    nc.scalar.activation(
                        uab[:],
                        s_ps[0:124, :, :],
                        ACTF.Abs,
                        bias=bias_abs[0:124, :],
                    )
                    scr = scrp.tile([124, 2, W], BF16, tag="scr")
                    nc.vector.scalar_tensor_tensor(
                        scr[:],
                        uab[:],
                        12.0,
                        spy_w[:, 2 * g: 2 * g + 2, :],
                        op0=ALU.is_gt,
                        op1=ALU.mult,
                        accum_out=stats_r[0:124, 2 * im + g: 2 * im + g + 1],
                    )

            # ---- tail phase 2 ----
            spy_tl = spp.tile([TAIL_NPW, IMGS, W], BF16, tag="spy")
            ln_i = nc.scalar.activation(
                spy_tl[:],
                ey_tl[:],
                ACTF.Ln,
                bias=1.0,
                accum_out=stats_sp[0:TAIL_NPW, IMGS: IMGS + 1],
            )
            for g in range(IMGS // 2):
                s_ps = psum.tile([128, 2, W], F32, tag="s")
                for j in range(2):
                    im = 2 * g + j
                    wgt = band_sb[:, 124: 124 + 16]
                    nc.tensor.matmul(
                        s_ps[0:TAIL_NPW, j, :], wgt, v_tl_t[:, im, 0:W],
                        start=True, stop=False,
                    )
                    nc.tensor.matmul(
                        s_ps[0:TAIL_NPW, j, :], wgt, v_tl_t[:, im, 1: W + 1],
                        start=False, stop=False,
                    )
                    nc.tensor.matmul(
                        s_ps[0:TAIL_NPW, j, :], wgt, t_tl[:, im, 4: W + 4],
                        start=False, stop=True,
                    )
                uab = uap.tile([TAIL_NPW, 2, W], BF16, tag="uab")
                nc.scalar.activation(
                    uab[:],
                    s_ps[0:TAIL_NPW, :, :],
                    ACTF.Abs,
                    bias=bias_abs[0:TAIL_NPW, :],
                )
                scr = scrp.tile([TAIL_NPW, 2, W], BF16, tag="scr")
                nc.vector.scalar_tensor_tensor(
                    scr[:],
                    uab[:],
                    12.0,
                    spy_tl[:, 2 * g: 2 * g + 2, :],
                    op0=ALU.is_gt,
                    op1=ALU.mult,
                    accum_out=stats_r[0:TAIL_NPW, 2 * IMGS + g: 2 * IMGS + g + 1],
                )

            nc.sync.dma_start(out_sp[:], stats_sp[:])
            nc.sync.dma_start(out_r[:], stats_r[:])

    nc.compile()
    nc.finalize()
    return nc


_NC = None


def _get_nc() -> bass.Bass:
    global _NC
    if _NC is None:
        _NC = _build_nc()
    return _NC


def _make_in_maps(pred: np.ndarray, target: np.ndarray) -> list[dict]:
    import ml_dtypes

    bf16 = ml_dtypes.bfloat16
    pred = np.ascontiguousarray(pred.reshape(B, H, W)).astype(bf16)
    target = target.reshape(B, H, W)
    tpad = np.zeros((B, TP, TP), dtype=bf16)
    tpad[:, PAD: PAD + H, PAD: PAD + W] = target.astype(bf16)
    bands = _make_bands().astype(bf16)

    # partition-major window stacks so every DMA is contiguous per partition
    WIN_IS = [0, 124, 248, 372, TAIL_IS]
    twin = np.empty((B, 128, NWIN, TP), dtype=bf16)
    for w, is_ in enumerate(WIN_IS):
        twin[:, :, w, :] = tpad[:, is_: is_ + 128, :]
    main = lambda a: np.ascontiguousarray(
        a[:, 0: 4 * 124, :].reshape(B, NMAIN, 124, a.shape[2]).transpose(0, 2, 1, 3)
    )
    xw = main(pred)
    taw = main(target.astype(bf16))
    xtail = np.ascontiguousarray(pred[:, TAIL_G0:, :])
    tatail = np.ascontiguousarray(target[:, TAIL_G0:, :].astype(bf16))

    in_maps = []
    for c in range(NCORES):
        sl = slice(c * IMGS, (c + 1) * IMGS)
        in_maps.append(
            {
                "twin": np.ascontiguousarray(twin[sl]),
                "xw": np.ascontiguousarray(xw[sl]),
                "taw": np.ascontiguousarray(taw[sl]),
                "xtail": xtail[sl],
                "tatail": tatail[sl],
                "band": bands,
            }
        )
    return in_maps


def _finish(results: list[dict]) -> np.ndarray:
    total = 0.0
    for res in results:
        total += 5.0 * np.sum(res["out_sp"], dtype=np.float64)
        total -= 4.0 * np.sum(res["out_r"], dtype=np.float64)
    mean = total / float(B * H * W)
    return np.asarray(np.float32(mean))


def kernel(pred: np.ndarray, target: np.ndarray, **run_kwargs) -> np.ndarray:
    pred = np.asarray(pred)
    target = np.asarray(target)
    nc = _get_nc()
    in_maps = _make_in_maps(pred, target)
    out = run_bass_kernel_spmd(nc, in_maps, core_ids=list(range(NCORES)), **run_kwargs)
    res = _finish(out.results)
    kernel.last_run = out
    return res



# revision 6
# speedup vs baseline: 1.4945x; 1.4945x over previous
"""V7: fp8/u8 inputs, single target load, 16-way DMA striping, balanced engines.

Key changes vs V6 (87.6us):
- target is loaded ONCE as 0/1 integer bytes (fp8-denormal coding: byte k
  == fp8 value k*2^-9, linear for k<=7), in 128-partition window stacks
  (twin).  The row-aligned view needed for the BCE sign flip is the same
  SBUF tile at partition offset 2 -- no second load (V6 shipped the
  target twice: 2.0MB bf16 taw + 2.6MB twin).
- pred is fp8 (e4m3, |x|<6 so quantization is ~2% RMS, statistically
  cancelled in the mean; rel err budget is 2e-2).  Total HBM in: 2.7MB
  vs 6.9MB.
- every big DMA has a 128-row outer dim: V6's 124-row xw/taw transfers
  striped across only 4 of 16 SDMA engines (largest divisor <=16 of 124
  is 4) -- that serialized 4MB through 4 engines at ~23GB/s each.
  Loads are split across both HWDGE rings (sync + scalar).
- box sum: v = t0+t2 as ONE u16 tensor_tensor add over packed bytes
  (integer coding -> no carries), then 3 fp8 matmuls per window with a
  shared 5-diag band; all window/tail cases are uniform [128,512] slabs
  (junk lanes self-silence: x junk = -240 -> softplus -> exactly 0).
- sign flip: mk = (t_u16 << 7) & M[p] (one op, per-partition M also
  masks junk lanes), y = mk ^ x.
- boundary mask: s==0 or 25  <=>  |s-12.5|>12 (scaled by 2^-9).  Split
  between ACT (Abs from PSUM) and DVE (d=s+c TT, q=d*d on Pool) to
  balance engines; products (mask*spy with accum) on DVE stt.
- exp/ln phases share table sets (2 loads total).
"""

import numpy as np

import concourse.bass as bass
import concourse.bacc as bacc_mod
import concourse.tile as tile
from concourse import mybir
from concourse.bass_utils import run_bass_kernel_spmd

F32 = mybir.dt.float32
BF16 = mybir.dt.bfloat16
FP8 = mybir.dt.float8e4
U16 = mybir.dt.uint16
ALU = mybir.AluOpType
ACTF = mybir.ActivationFunctionType

B, H, W = 32, 512, 512
NCORES = 8
IMGS = B // NCORES          # 4 images per core
PAD = 2
TP = H + 2 * PAD            # 516
NWIN = 5
WIN_IS = [0, 124, 248, 372, 388]   # tpad start row of each window
SC = 2.0 ** -9              # denormal coding scale of the 0/1 target bytes

# stats layout: col i in [0,4): ln-accum (A_i); col 4+3i+j: R accums
NSTAT = 4 + 3 * IMGS


def _ap3(t, off, dims):
    return bass.AP(t, off, dims)


def _build_nc() -> bass.Bass:
    nc = bacc_mod.Bacc(trn_type="TRN2")

    twin = nc.dram_tensor("twin", [IMGS, 128, NWIN, TP], FP8, kind="ExternalInput")
    xpk = nc.dram_tensor("xpk", [IMGS, 128, NWIN, W], FP8, kind="ExternalInput")
    band = nc.dram_tensor("band", [128, 128], FP8, kind="ExternalInput")
    mcols = nc.dram_tensor("mcols", [128, 2], U16, kind="ExternalInput")
    stats = nc.dram_tensor("stats", [128, NSTAT], F32, kind="ExternalOutput")

    with tile.TileContext(nc) as tc:
        with (
            tc.tile_pool(name="singles", bufs=1) as singles,
            tc.tile_pool(name="tin", bufs=3) as tin,
            tc.tile_pool(name="xin", bufs=2) as xin,
            tc.tile_pool(name="vp", bufs=2) as vp,
            tc.tile_pool(name="mkp", bufs=2) as mkp,
            tc.tile_pool(name="yp", bufs=2) as yp,
            tc.tile_pool(name="eyp", bufs=4) as eyp,
            tc.tile_pool(name="spp", bufs=2) as spp,
            tc.tile_pool(name="uabp", bufs=3) as uabp,
            tc.tile_pool(name="dp", bufs=3) as dp,
            tc.tile_pool(name="qp", bufs=3) as qp,
            tc.tile_pool(name="scrp", bufs=3) as scrp,
            tc.tile_pool(name="ps2", bufs=3, space="PSUM") as ps2,
            tc.tile_pool(name="ps1", bufs=2, space="PSUM") as ps1,
        ):
            band_sb = singles.tile([128, 128], FP8)
            m_sb = singles.tile([128, 2], U16)
            nc.sync.dma_start(band_sb[:], band[:, :])
            nc.sync.dma_start(m_sb[:], mcols[:, :])

            stats_sb = singles.tile([128, NSTAT], F32)
            nc.vector.memset(stats_sb[:], 0.0)
            nbias = singles.tile([128, 2, W], BF16)
            nc.vector.memset(nbias[:], -12.5 * SC)
            bias_abs = singles.tile([128, 1], F32)
            nc.gpsimd.memset(bias_abs[:], -12.5 * SC)

            t_sb = [None] * IMGS
            x_sb = [None] * IMGS
            v_sb = [None] * IMGS
            y_sb = [None] * IMGS
            ey_sb = [None] * IMGS

            # ---- input DMAs: twin on the sync ring, xpk on the scalar ring
            for i in range(IMGS):
                t_sb[i] = tin.tile([128, NWIN, TP], FP8, tag="t", name=f"t{i}")
                nc.sync.dma_start(
                    t_sb[i][:],
                    _ap3(twin, i * 128 * NWIN * TP,
                         [[NWIN * TP, 128], [TP, NWIN], [1, TP]]),
                )
                x_sb[i] = xin.tile([128, NWIN, W], FP8, tag="x", name=f"x{i}")
                nc.scalar.dma_start(
                    x_sb[i][:],
                    _ap3(xpk, i * 128 * NWIN * W,
                         [[NWIN * W, 128], [W, NWIN], [1, W]]),
                )

            # ---- phase 1 per image: v, mk, y (DVE), exp (ACT)
            for i in range(IMGS):
                v_sb[i] = vp.tile([128, NWIN, TP - 2], FP8, tag="v", name=f"v{i}")
                nc.vector.tensor_tensor(
                    v_sb[i][:].bitcast(U16),
                    t_sb[i][:, :, 0:TP - 2].bitcast(U16),
                    t_sb[i][:, :, 2:TP].bitcast(U16),
                    op=ALU.add,
                )
                mk = mkp.tile([128, NWIN, W], FP8, tag="mk")
                nc.vector.tensor_scalar(
                    mk[:, 0:NWIN - 1, :].bitcast(U16),
                    t_sb[i][:, 0:NWIN - 1, 2:2 + W].bitcast(U16),
                    7,
                    m_sb[:, 0:1],
                    op0=ALU.logical_shift_left,
                    op1=ALU.bitwise_and,
                )
                nc.vector.tensor_scalar(
                    mk[:, NWIN - 1:NWIN, :].bitcast(U16),
                    t_sb[i][:, NWIN - 1:NWIN, 2:2 + W].bitcast(U16),
                    7,
                    m_sb[:, 1:2],
                    op0=ALU.logical_shift_left,
                    op1=ALU.bitwise_and,
                )
                y_sb[i] = yp.tile([128, NWIN, W], FP8, tag="y", name=f"y{i}")
                nc.vector.tensor_tensor(
                    y_sb[i][:].bitcast(U16),
                    mk[:].bitcast(U16),
                    x_sb[i][:].bitcast(U16),
                    op=ALU.bitwise_xor,
                )
                ey_sb[i] = eyp.tile([128, NWIN, W], F32, tag="ey", name=f"ey{i}")
                nc.scalar.activation(ey_sb[i][:], y_sb[i][:], ACTF.Exp)

            # ---- box matmuls + mask precursors
            # uab_like[(i, g)] = (tile, threshold) for the product stt
            uab_like = {}
            for i in range(IMGS):
                for g in range(2):          # window pairs (0,1) and (2,3)
                    s2 = ps2.tile([128, 2, W], F32, tag="s2")
                    for j in range(2):
                        w = 2 * g + j
                        nc.tensor.matmul(
                            s2[:, j, :], band_sb[:], v_sb[i][:, w, 0:W],
                            start=True, stop=False)
                        nc.tensor.matmul(
                            s2[:, j, :], band_sb[:], v_sb[i][:, w, 1:W + 1],
                            start=False, stop=False)
                        nc.tensor.matmul(
                            s2[:, j, :], band_sb[:], t_sb[i][:, w, 4:W + 4],
                            start=False, stop=True)
                    if g == 0:
                        # ACT path: uab = |s - 12.5*SC|
                        uab = uabp.tile([128, 2, W], BF16, tag="uab")
                        nc.scalar.activation(
                            uab[:], s2[:], ACTF.Abs, bias=bias_abs[:])
                        uab_like[(i, g)] = (uab, 12.0 * SC)
                    else:
                        # DVE+Pool path: d = s + (-12.5*SC); q = d*d
                        d = dp.tile([128, 2, W], BF16, tag="d")
                        nc.vector.tensor_tensor(d[:], s2[:], nbias[:], op=ALU.add)
                        q = qp.tile([128, 2, W], BF16, tag="q")
                        nc.gpsimd.tensor_tensor(q[:], d[:], d[:], op=ALU.mult)
                        uab_like[(i, g)] = (q, 144.0 * SC * SC)

                # tail window (w=4), single 512 slab, DVE+Pool path
                s1 = ps1.tile([128, W], F32, tag="s1")
                nc.tensor.matmul(
                    s1[:], band_sb[:], v_sb[i][:, 4, 0:W],
                    start=True, stop=False)
                nc.tensor.matmul(
                    s1[:], band_sb[:], v_sb[i][:, 4, 1:W + 1],
                    start=False, stop=False)
                nc.tensor.matmul(
                    s1[:], band_sb[:], t_sb[i][:, 4, 4:W + 4],
                    start=False, stop=True)
                d1 = dp.tile([128, W], BF16, tag="d1")
                nc.vector.tensor_tensor(d1[:], s1[:], nbias[:, 0, :], op=ALU.add)
                q1 = qp.tile([128, W], BF16, tag="q1")
                nc.gpsimd.tensor_tensor(q1[:], d1[:], d1[:], op=ALU.mult)
                uab_like[(i, 2)] = (q1, 144.0 * SC * SC)

            # ---- phase 2 per image: ln (+accum) then masked products
            for i in range(IMGS):
                spy = spp.tile([128, NWIN, W], BF16, tag="spy")
                nc.scalar.activation(
                    spy[:], ey_sb[i][:], ACTF.Ln, bias=1.0,
                    accum_out=stats_sb[:, i:i + 1],
                )
                for g in range(2):
                    src, thr = uab_like[(i, g)]
                    scr = scrp.tile([128, 2, W], BF16, tag="scr")
                    nc.vector.scalar_tensor_tensor(
                        scr[:], src[:], thr, spy[:, 2 * g:2 * g + 2, :],
                        op0=ALU.is_gt, op1=ALU.mult,
                        accum_out=stats_sb[:, 4 + 3 * i + g:5 + 3 * i + g],
                    )
                src, thr = uab_like[(i, 2)]
                scr1 = scrp.tile([128, W], BF16, tag="scr1")
                nc.vector.scalar_tensor_tensor(
                    scr1[:], src[:], thr, spy[:, 4, :],
                    op0=ALU.is_gt, op1=ALU.mult,
                    accum_out=stats_sb[:, 6 + 3 * i:7 + 3 * i],
                )

            nc.sync.dma_start(stats[:, :], stats_sb[:])

    nc.compile()
    nc.finalize()
    return nc


_NC = None


def _get_nc() -> bass.Bass:
    global _NC
    if _NC is None:
        _NC = _build_nc()
    return _NC


def _make_in_maps(pred: np.ndarray, target: np.ndarray) -> list[dict]:
    import ml_dtypes

    fp8 = ml_dtypes.float8_e4m3fn
    x8 = pred.reshape(B, H, W).astype(fp8)
    t_u8 = target.reshape(B, H, W).astype(np.uint8)

    tpad = np.zeros((B, TP, TP), dtype=np.uint8)
    tpad[:, PAD:PAD + H, PAD:PAD + W] = t_u8
    rows = np.asarray(WIN_IS)[:, None] + np.arange(128)[None, :]  # [5, 128]
    twin = np.ascontiguousarray(
        tpad[:, rows, :].transpose(0, 2, 1, 3)).view(fp8)          # [B,128,5,516]

    xpk = np.full((B, 128, NWIN, W), -240.0, dtype=fp8)
    for g in range(4):
        xpk[:, 2:126, g, :] = x8[:, 124 * g:124 * g + 124, :]
    xpk[:, 110:126, 4, :] = x8[:, 496:512, :]

    band = np.zeros((128, 128), dtype=np.float32)
    for m in range(2, 126):
        band[m - 2:m + 3, m] = 1.0
    band = band.astype(fp8)

    mcols = np.zeros((128, 2), dtype=np.uint16)
    mcols[2:126, 0] = 0x8080
    mcols[110:126, 1] = 0x8080

    in_maps = []
    for c in range(NCORES):
        sl = slice(c * IMGS, (c + 1) * IMGS)
        in_maps.append(
            {
                "twin": np.ascontiguousarray(twin[sl]),
                "xpk": np.ascontiguousarray(xpk[sl]),
                "band": band,
                "mcols": mcols,
            }
        )
    return in_maps


def _finish(results: list[dict]) -> np.ndarray:
    total = 0.0
    for res in results:
        st = res["stats"].astype(np.float64)
        total += 5.0 * st[:, 0:IMGS].sum()
        total -= 4.0 * st[:, IMGS:].sum()
    mean = total / float(B * H * W)
    return np.asarray(np.float32(mean))


def kernel(pred: np.ndarray, target: np.ndarray, **run_kwargs) -> np.ndarray:
    pred = np.asarray(pred)
    target = np.asarray(target)
    nc = _get_nc()
    in_maps = _make_in_maps(pred, target)
    out = run_bass_kernel_spmd(nc, in_maps, core_ids=list(range(NCORES)), **run_kwargs)
    res = _finish(out.results)
    kernel.last_run = out
    return res


# revision 7
# speedup vs baseline: 1.6096x; 1.0770x over previous
"""V8: fp8/u8 inputs, presigned pred, unified square-mask, one product/img.

Lineage: V6 (87.6us, bf16, dual target load) -> V7 (57.9us, fp8 + single
target load + 16-way DMA striping) -> V8.

V8 changes vs V7 (from the V7 trace: DVE busy 34us = wall, ACT 29us,
Pool slow at 0.55 elem/cyc, stt products run at 1x, 12us of DVE
semaphore overhead from fine-grained ops):
- pred ships PRE-SIGNED (host XORs the target's sign bit into the fp8
  byte -- a bijective re-encoding of (pred, target); bce = softplus(y)
  with y = (1-2t)x).  Drops the mk/xor DVE ops and their sync.
- mask precursor unified: Q = (s - 12.5*SC)^2 in ONE [128,5,512] bf16
  tile per image; windows 0-1 computed by ACT Square straight from PSUM
  (square is a filler fn in every table set -> no extra table load),
  windows 2-4 by DVE add + Pool mult.  One stt product per image
  (FD 2560) instead of three -> fewer 1x-rate stt passes and fewer
  semaphores.
- all input DMAs on the sync HWDGE ring (one ring already spreads over
  all 16 SDMA engines; frees the scalar engine for pure ACT work),
  interleaved twin/ypk per image so image 0 lands first.
- target still loaded once as 0/1 integer bytes == fp8 denormals
  (k * 2^-9, exactly linear), one u16 packed add for the column pair
  sums, 3 fp8 matmuls per 128-row window against a shared 5-diag band;
  thresholds scale by 2^-9 exactly.
"""

import numpy as np

import concourse.bass as bass
import concourse.bacc as bacc_mod
import concourse.tile as tile
from concourse import mybir
from concourse.bass_utils import run_bass_kernel_spmd

F32 = mybir.dt.float32
BF16 = mybir.dt.bfloat16
FP8 = mybir.dt.float8e4
U16 = mybir.dt.uint16
ALU = mybir.AluOpType
ACTF = mybir.ActivationFunctionType

B, H, W = 32, 512, 512
NCORES = 8
IMGS = B // NCORES          # 4 images per core
PAD = 2
TP = H + 2 * PAD            # 516
NWIN = 5
WIN_IS = [0, 124, 248, 372, 388]   # tpad start row of each window
SC = 2.0 ** -9              # denormal coding scale of the 0/1 target bytes
QTHR = 144.0 * SC * SC      # (s-12.5)^2 > 144  <=>  s in {0, 25}

# stats: col i in [0,4): ln accum A_i; col 4+i: masked-product accum R_i
NSTAT = 2 * IMGS

# which (img, pair-g0) masks go to ACT Square (rest via DVE+Pool)
ACT_MASK_IMGS = (0, 1)


def _ap3(t, off, dims):
    return bass.AP(t, off, dims)


def _build_nc() -> bass.Bass:
    nc = bacc_mod.Bacc(trn_type="TRN2")

    twin = nc.dram_tensor("twin", [IMGS, 128, NWIN, TP], FP8, kind="ExternalInput")
    ypk = nc.dram_tensor("ypk", [IMGS, 128, NWIN, W], FP8, kind="ExternalInput")
    band = nc.dram_tensor("band", [128, 128], FP8, kind="ExternalInput")
    stats = nc.dram_tensor("stats", [128, NSTAT], F32, kind="ExternalOutput")

    with tile.TileContext(nc) as tc:
        with (
            tc.tile_pool(name="singles", bufs=1) as singles,
            tc.tile_pool(name="tin", bufs=3) as tin,
            tc.tile_pool(name="yin", bufs=2) as yin,
            tc.tile_pool(name="vp", bufs=2) as vp,
            tc.tile_pool(name="eyp", bufs=4) as eyp,
            tc.tile_pool(name="qmp", bufs=2) as qmp,
            tc.tile_pool(name="spp", bufs=2) as spp,
            tc.tile_pool(name="dp", bufs=3) as dp,
            tc.tile_pool(name="scrp", bufs=2) as scrp,
            tc.tile_pool(name="ps2", bufs=3, space="PSUM") as ps2,
            tc.tile_pool(name="ps1", bufs=2, space="PSUM") as ps1,
        ):
            band_sb = singles.tile([128, 128], FP8)
            nc.sync.dma_start(band_sb[:], band[:, :])

            stats_sb = singles.tile([128, NSTAT], F32)
            nc.vector.memset(stats_sb[:], 0.0)
            nbias = singles.tile([128, 2, W], BF16)
            nc.vector.memset(nbias[:], -12.5 * SC)
            bias_sq = singles.tile([128, 1], F32)
            nc.gpsimd.memset(bias_sq[:], -12.5 * SC)

            t_sb = [None] * IMGS
            y_sb = [None] * IMGS
            v_sb = [None] * IMGS
            ey_sb = [None] * IMGS
            q_sb = [None] * IMGS

            # ---- input DMAs, all on the sync ring, image-major order
            for i in range(IMGS):
                t_sb[i] = tin.tile([128, NWIN, TP], FP8, tag="t", name=f"t{i}")
                nc.sync.dma_start(
                    t_sb[i][:],
                    _ap3(twin, i * 128 * NWIN * TP,
                         [[NWIN * TP, 128], [TP, NWIN], [1, TP]]),
                )
                y_sb[i] = yin.tile([128, NWIN, W], FP8, tag="y", name=f"y{i}")
                nc.sync.dma_start(
                    y_sb[i][:],
                    _ap3(ypk, i * 128 * NWIN * W,
                         [[NWIN * W, 128], [W, NWIN], [1, W]]),
                )

            # ---- phase 1 per image: v (DVE), exp (ACT)
            for i in range(IMGS):
                v_sb[i] = vp.tile([128, NWIN, TP - 2], FP8, tag="v", name=f"v{i}")
                nc.vector.tensor_tensor(
                    v_sb[i][:].bitcast(U16),
                    t_sb[i][:, :, 0:TP - 2].bitcast(U16),
                    t_sb[i][:, :, 2:TP].bitcast(U16),
                    op=ALU.add,
                )
                ey_sb[i] = eyp.tile([128, NWIN, W], F32, tag="ey", name=f"ey{i}")
                nc.scalar.activation(ey_sb[i][:], y_sb[i][:], ACTF.Exp)

            # ---- box matmuls + unified mask precursor Q = (s-12.5*SC)^2
            for i in range(IMGS):
                q_sb[i] = qmp.tile([128, NWIN, W], BF16, tag="q", name=f"q{i}")
                for g in range(2):          # window pairs (0,1) and (2,3)
                    s2 = ps2.tile([128, 2, W], F32, tag="s2")
                    for j in range(2):
                        w = 2 * g + j
                        nc.tensor.matmul(
                            s2[:, j, :], band_sb[:], v_sb[i][:, w, 0:W],
                            start=True, stop=False)
                        nc.tensor.matmul(
                            s2[:, j, :], band_sb[:], v_sb[i][:, w, 1:W + 1],
                            start=False, stop=False)
                        nc.tensor.matmul(
                            s2[:, j, :], band_sb[:], t_sb[i][:, w, 4:W + 4],
                            start=False, stop=True)
                    if g == 0 and i in ACT_MASK_IMGS:
                        nc.scalar.activation(
                            q_sb[i][:, 0:2, :], s2[:], ACTF.Square,
                            bias=bias_sq[:])
                    else:
                        d = dp.tile([128, 2, W], BF16, tag="d")
                        nc.vector.tensor_tensor(d[:], s2[:], nbias[:], op=ALU.add)
                        nc.gpsimd.tensor_tensor(
                            q_sb[i][:, 2 * g:2 * g + 2, :], d[:], d[:],
                            op=ALU.mult)

                # tail window (w=4)
                s1 = ps1.tile([128, W], F32, tag="s1")
                nc.tensor.matmul(
                    s1[:], band_sb[:], v_sb[i][:, 4, 0:W],
                    start=True, stop=False)
                nc.tensor.matmul(
                    s1[:], band_sb[:], v_sb[i][:, 4, 1:W + 1],
                    start=False, stop=False)
                nc.tensor.matmul(
                    s1[:], band_sb[:], t_sb[i][:, 4, 4:W + 4],
                    start=False, stop=True)
                d1 = dp.tile([128, W], BF16, tag="d1")
                nc.vector.tensor_tensor(d1[:], s1[:], nbias[:, 0, :], op=ALU.add)
                nc.gpsimd.tensor_tensor(
                    q_sb[i][:, 4, :], d1[:], d1[:], op=ALU.mult)

            # ---- phase 2 per image: ln (+accum), one masked product
            for i in range(IMGS):
                spy = spp.tile([128, NWIN, W], BF16, tag="spy")
                nc.scalar.activation(
                    spy[:], ey_sb[i][:], ACTF.Ln, bias=1.0,
                    accum_out=stats_sb[:, i:i + 1],
                )
                scr = scrp.tile([128, NWIN, W], BF16, tag="scr")
                nc.vector.scalar_tensor_tensor(
                    scr[:], q_sb[i][:], QTHR, spy[:],
                    op0=ALU.is_gt, op1=ALU.mult,
                    accum_out=stats_sb[:, IMGS + i:IMGS + i + 1],
                )

            nc.sync.dma_start(stats[:, :], stats_sb[:])

    nc.compile()
    nc.finalize()
    return nc


_NC = None


def _get_nc() -> bass.Bass:
    global _NC
    if _NC is None:
        _NC = _build_nc()
    return _NC


def _make_in_maps(pred: np.ndarray, target: np.ndarray) -> list[dict]:
    import ml_dtypes

    fp8 = ml_dtypes.float8_e4m3fn
    x8 = pred.reshape(B, H, W).astype(fp8)
    t_u8 = target.reshape(B, H, W).astype(np.uint8)
    # presigned pred: flip the fp8 sign bit where target == 1 (bit-exact
    # equivalent of the on-device XOR in V7)
    ysig = (x8.view(np.uint8) ^ (t_u8 << 7)).view(fp8)

    tpad = np.zeros((B, TP, TP), dtype=np.uint8)
    tpad[:, PAD:PAD + H, PAD:PAD + W] = t_u8
    rows = np.asarray(WIN_IS)[:, None] + np.arange(128)[None, :]  # [5, 128]
    twin = np.ascontiguousarray(
        tpad[:, rows, :].transpose(0, 2, 1, 3)).view(fp8)          # [B,128,5,516]

    ypk = np.full((B, 128, NWIN, W), -240.0, dtype=fp8)
    for g in range(4):
        ypk[:, 2:126, g, :] = ysig[:, 124 * g:124 * g + 124, :]
    ypk[:, 110:126, 4, :] = ysig[:, 496:512, :]

    band = np.zeros((128, 128), dtype=np.float32)
    for m in range(2, 126):
        band[m - 2:m + 3, m] = 1.0
    band = band.astype(fp8)

    in_maps = []
    for c in range(NCORES):
        sl = slice(c * IMGS, (c + 1) * IMGS)
        in_maps.append(
            {
                "twin": np.ascontiguousarray(twin[sl]),
                "ypk": np.ascontiguousarray(ypk[sl]),
                "band": band,
            }
        )
    return in_maps


def _finish(results: list[dict]) -> np.ndarray:
    total = 0.0
    for res in results:
        st = res["stats"].astype(np.float64)
        total += 5.0 * st[:, 0:IMGS].sum()
        total -= 4.0 * st[:, IMGS:].sum()
    mean = total / float(B * H * W)
    return np.asarray(np.float32(mean))


def kernel(pred: np.ndarray, target: np.ndarray, **run_kwargs) -> np.ndarray:
    pred = np.asarray(pred)
    target = np.asarray(target)
    nc = _get_nc()
    in_maps = _make_in_maps(pred, target)
    out = run_bass_kernel_spmd(nc, in_maps, core_ids=list(range(NCORES)), **run_kwargs)
    res = _finish(out.results)
    kernel.last_run = out
    return res


# revision 10
# speedup vs baseline: 1.8037x; 1.1206x over previous
"""V9: one packed DMA per image, no Pool compute, DVE d^2 masks, 1 product/img.

Lineage: V6 (87.6us, bf16, dual target load) -> V7 (57.9us, fp8 + single
target load + 16-way DMA striping) -> V8 (53.8us, presigned pred, unified
square mask) -> V9.

V9 changes vs V8 (from the V8 trace: Pool TENSOR_TENSOR ran at ~0.4
elem/cyc with 0.8us drains and pushed the last masked product to 49us;
10 serial DMA issues at ~0.6us each delayed image 0's data to ~11us):
- each image ships as ONE packed [128, 5, 516+512] fp8 tensor: per
  window, 516 target bytes (0/1 integer coding == fp8 denormal k*2^-9)
  followed by 512 presigned-pred bytes.  4 input DMAs instead of 8,
  image 0 lands ~2us earlier.
- Pool does no elementwise work: q = d*d runs on DVE at 2x (bf16);
  windows 0-1 of every image still use ACT Square straight from PSUM
  (square is a filler fn in every ACT table set -> no extra load).
- one scalar_tensor_tensor product per image ((Q > thr) * spy with
  accum), threshold uniform at (12*2^-9)^2.
- bce = softplus(y) with y = presigned pred; sum via ln(1+exp(y)) accum;
  box sum via one u16 packed add (v = t0+t2) + 3 fp8 matmuls per
  128-row window against a shared 5-diagonal band.
"""

import numpy as np

import concourse.bass as bass
import concourse.bacc as bacc_mod
import concourse.tile as tile
from concourse import mybir
from concourse.bass_utils import run_bass_kernel_spmd

F32 = mybir.dt.float32
BF16 = mybir.dt.bfloat16
FP8 = mybir.dt.float8e4
U16 = mybir.dt.uint16
ALU = mybir.AluOpType
ACTF = mybir.ActivationFunctionType

B, H, W = 32, 512, 512
NCORES = 8
IMGS = B // NCORES          # 4 images per core
PAD = 2
TP = H + 2 * PAD            # 516
NWIN = 5
PKC = TP + W                # 1028 packed bytes per (partition, window)
WIN_IS = [0, 124, 248, 372, 388]   # tpad start row of each window
SC = 2.0 ** -9              # denormal coding scale of the 0/1 target bytes
QTHR = 144.0 * SC * SC      # (s-12.5)^2 > 144  <=>  s in {0, 25}

# stats: col i in [0,4): ln accum A_i; col 4+i: masked-product accum R_i
NSTAT = 2 * IMGS


def _ap3(t, off, dims):
    return bass.AP(t, off, dims)


def _build_nc() -> bass.Bass:
    nc = bacc_mod.Bacc(trn_type="TRN2")

    pk = nc.dram_tensor("pk", [IMGS, 128, NWIN, PKC], FP8, kind="ExternalInput")
    band = nc.dram_tensor("band", [128, 128], FP8, kind="ExternalInput")
    stats = nc.dram_tensor("stats", [128, NSTAT], F32, kind="ExternalOutput")

    with tile.TileContext(nc) as tc:
        with (
            tc.tile_pool(name="singles", bufs=1) as singles,
            tc.tile_pool(name="pkin", bufs=3) as pkin,
            tc.tile_pool(name="vp", bufs=2) as vp,
            tc.tile_pool(name="eyp", bufs=4) as eyp,
            tc.tile_pool(name="qmp", bufs=2) as qmp,
            tc.tile_pool(name="spp", bufs=2) as spp,
            tc.tile_pool(name="dp", bufs=3) as dp,
            tc.tile_pool(name="scrp", bufs=2) as scrp,
            tc.tile_pool(name="ps2", bufs=3, space="PSUM") as ps2,
            tc.tile_pool(name="ps1", bufs=2, space="PSUM") as ps1,
        ):
            stats_sb = singles.tile([128, NSTAT], F32)
            nc.vector.memset(stats_sb[:], 0.0)
            nbias = singles.tile([128, 2, W], BF16)
            nc.vector.memset(nbias[:], -12.5 * SC)
            bias_sq = singles.tile([128, 1], F32)
            nc.gpsimd.memset(bias_sq[:], -12.5 * SC)

            pk_sb = [None] * IMGS
            v_sb = [None] * IMGS
            ey_sb = [None] * IMGS
            q_sb = [None] * IMGS

            # ---- input DMAs on the sync ring; image 0 first, band mid-queue
            band_sb = singles.tile([128, 128], FP8)
            for i in range(IMGS):
                pk_sb[i] = pkin.tile([128, NWIN, PKC], FP8, tag="pk",
                                     name=f"pk{i}")
                nc.sync.dma_start(
                    pk_sb[i][:],
                    _ap3(pk, i * 128 * NWIN * PKC,
                         [[NWIN * PKC, 128], [PKC, NWIN], [1, PKC]]),
                )
                if i == 1:
                    nc.sync.dma_start(band_sb[:], band[:, :])

            # ---- phase 1 per image: v (DVE), exp (ACT)
            for i in range(IMGS):
                tpk = pk_sb[i]
                v_sb[i] = vp.tile([128, NWIN, TP - 2], FP8, tag="v", name=f"v{i}")
                nc.vector.tensor_tensor(
                    v_sb[i][:].bitcast(U16),
                    tpk[:, :, 0:TP - 2].bitcast(U16),
                    tpk[:, :, 2:TP].bitcast(U16),
                    op=ALU.add,
                )
                ey_sb[i] = eyp.tile([128, NWIN, W], F32, tag="ey", name=f"ey{i}")
                nc.scalar.activation(ey_sb[i][:], tpk[:, :, TP:PKC], ACTF.Exp)

            # ---- box matmuls + unified mask precursor Q = (s-12.5*SC)^2
            for i in range(IMGS):
                tpk = pk_sb[i]
                q_sb[i] = qmp.tile([128, NWIN, W], BF16, tag="q", name=f"q{i}")
                for g in range(2):          # window pairs (0,1) and (2,3)
                    s2 = ps2.tile([128, 2, W], F32, tag="s2")
                    for j in range(2):
                        w = 2 * g + j
                        nc.tensor.matmul(
                            s2[:, j, :], band_sb[:], v_sb[i][:, w, 0:W],
                            start=True, stop=False)
                        nc.tensor.matmul(
                            s2[:, j, :], band_sb[:], v_sb[i][:, w, 1:W + 1],
                            start=False, stop=False)
                        nc.tensor.matmul(
                            s2[:, j, :], band_sb[:], tpk[:, w, 4:W + 4],
                            start=False, stop=True)
                    if g == 0:
                        nc.scalar.activation(
                            q_sb[i][:, 0:2, :], s2[:], ACTF.Square,
                            bias=bias_sq[:])
                    else:
                        d = dp.tile([128, 2, W], BF16, tag="d")
                        nc.vector.tensor_tensor(d[:], s2[:], nbias[:], op=ALU.add)
                        nc.vector.tensor_tensor(
                            q_sb[i][:, 2:4, :], d[:], d[:], op=ALU.mult)

                # tail window (w=4)
                s1 = ps1.tile([128, W], F32, tag="s1")
                nc.tensor.matmul(
                    s1[:], band_sb[:], v_sb[i][:, 4, 0:W],
                    start=True, stop=False)
                nc.tensor.matmul(
                    s1[:], band_sb[:], v_sb[i][:, 4, 1:W + 1],
                    start=False, stop=False)
                nc.tensor.matmul(
                    s1[:], band_sb[:], tpk[:, 4, 4:W + 4],
                    start=False, stop=True)
                d1 = dp.tile([128, W], BF16, tag="d1")
                nc.vector.tensor_tensor(d1[:], s1[:], nbias[:, 0, :], op=ALU.add)
                nc.vector.tensor_tensor(
                    q_sb[i][:, 4, :], d1[:], d1[:], op=ALU.mult)

            # ---- phase 2 per image: ln (+accum), one masked product
            for i in range(IMGS):
                spy = spp.tile([128, NWIN, W], BF16, tag="spy")
                nc.scalar.activation(
                    spy[:], ey_sb[i][:], ACTF.Ln, bias=1.0,
                    accum_out=stats_sb[:, i:i + 1],
                )
                scr = scrp.tile([128, NWIN, W], BF16, tag="scr")
                nc.vector.scalar_tensor_tensor(
                    scr[:], q_sb[i][:], QTHR, spy[:],
                    op0=ALU.is_gt, op1=ALU.mult,
                    accum_out=stats_sb[:, IMGS + i:IMGS + i + 1],
                )

            nc.sync.dma_start(stats[:, :], stats_sb[:])

    nc.compile()
    nc.finalize()
    return nc


_NC = None


def _get_nc() -> bass.Bass:
    global _NC
    if _NC is None:
        _NC = _build_nc()
    return _NC


def _make_in_maps(pred: np.ndarray, target: np.ndarray) -> list[dict]:
    import ml_dtypes

    fp8 = ml_dtypes.float8_e4m3fn
    x8 = pred.reshape(B, H, W).astype(fp8)
    t_u8 = target.reshape(B, H, W).astype(np.uint8)
    # presigned pred: flip the fp8 sign bit where target == 1 (bit-exact
    # equivalent of an on-device XOR)
    ysig = (x8.view(np.uint8) ^ (t_u8 << 7))            # uint8

    tpad = np.zeros((B, TP, TP), dtype=np.uint8)
    tpad[:, PAD:PAD + H, PAD:PAD + W] = t_u8
    rows = np.asarray(WIN_IS)[:, None] + np.arange(128)[None, :]  # [5, 128]
    twin = tpad[:, rows, :].transpose(0, 2, 1, 3)        # [B,128,5,516] u8

    junk = np.asarray(-240.0, dtype=fp8).view(np.uint8).item()   # 0xF7
    ypk = np.full((B, 128, NWIN, W), junk, dtype=np.uint8)
    for g in range(4):
        ypk[:, 2:126, g, :] = ysig[:, 124 * g:124 * g + 124, :]
    ypk[:, 110:126, 4, :] = ysig[:, 496:512, :]

    pk = np.concatenate([twin, ypk], axis=3)             # [B,128,5,1028] u8
    pk = np.ascontiguousarray(pk).view(fp8)

    band = np.zeros((128, 128), dtype=np.float32)
    for m in range(2, 126):
        band[m - 2:m + 3, m] = 1.0
    band = band.astype(fp8)

    in_maps = []
    for c in range(NCORES):
        sl = slice(c * IMGS, (c + 1) * IMGS)
        in_maps.append(
            {
                "pk": np.ascontiguousarray(pk[sl]),
                "band": band,
            }
        )
    return in_maps


def _finish(results: list[dict]) -> np.ndarray:
    total = 0.0
    for res in results:
        st = res["stats"].astype(np.float64)
        total += 5.0 * st[:, 0:IMGS].sum()
        total -= 4.0 * st[:, IMGS:].sum()
    mean = total / float(B * H * W)
    return np.asarray(np.float32(mean))


def kernel(pred: np.ndarray, target: np.ndarray, **run_kwargs) -> np.ndarray:
    pred = np.asarray(pred)
    target = np.asarray(target)
    nc = _get_nc()
    in_maps = _make_in_maps(pred, target)
    out = run_bass_kernel_spmd(nc, in_maps, core_ids=list(range(NCORES)), **run_kwargs)
    res = _finish(out.results)
    kernel.last_run = out
    return res
